# revision 12
# baseline (speedup 1.0000x reference)
"""Attentional Factorization Machine — Trainium2 Bass kernel (8 NeuronCores).

Sharding: data-parallel over batch (2048/8 = 256 per core, as 2 groups of 128).

v2 dataflow (bf16 interaction branch, fp32 linear branch):
  - Host packs an augmented bf16 table [500000, 66]: cols 0-63 = emb (bf16),
    cols 64-65 = raw fp32 bits of lin_w (viewed on-chip via AP bitcast).
  - Per 128-batch group: 50 indirect-DMA row gathers -> gt [128, 50*66] bf16.
  - Linear term: fp32 bitcast view of the lin columns, free-axis reduce.
  - PE transposes (bf16) -> factors^T fact [128 = (d, batch-half),
    50*64 = (field, batch64)].
  - Pair blocks i: inter = f_i * f_j on DVE in (j, b) order -> 2-byte packed
    last dim engages the DVE 2x perf mode.
  - Flat pair chunking: global pair index p = 0..1224, chunks of 8 pairs,
    windows of 16 (one 2-bank PSUM tile). Segments split at block borders.
  - mm1: h' = (W1 diag|w2|)^T inter per half, col-tiled streams into window
    tile hp; pair-scalar streams (M=32, zero-padded stationaries so drains
    see fully-written partitions) run on cell-disjoint array columns into
    slab sl: sum1 (64,0)->sl[0:32], lgt1 (64,32)->sl[32:64],
    sum0 (0,64)->sl[64:96], lgt0 (0,96)->sl[96:128]; logits stream the
    PREVIOUS window's hs (software pipelining).
  - Drains: relu(h'+b1') -> hs bf16; slab copy -> fp32 (ACT/DVE mix); strips
    stream b-outer so compaction to batch-major is contiguous-run SBUF DMA.
  - Softmax over pairs; y = attended + linear. b2 dropped (constant logit
    shift cancels in softmax).
"""

import sys

for _p in ("/opt/trn_rl_repo",):
    if _p not in sys.path:
        sys.path.insert(0, _p)

import numpy as np
import ml_dtypes

import concourse.bass as bass
from concourse import bacc
import concourse.mybir as mybir
from concourse.tile import TileContext
from concourse.masks import make_identity
from concourse.bass_utils import run_bass_kernel_spmd

F = 50
D = 64
CARD = 10000
B = 2048
NCORES = 8
BPC = B // NCORES          # 256 batches per core
G = 2                      # groups of 128 per core
P = F * (F - 1) // 2       # 1225 pairs
ROW = D + 2                # bf16 row: 64 emb + 2 cols of lin_w fp32 bits
WPAIR = 16                 # pairs per window (= 1024 cols = 2 PSUM banks)
FP32 = mybir.dt.float32
BF16 = mybir.dt.bfloat16

# Segments: maximal runs of pairs within one block AND one 8-pair chunk.
# (block_i, j0, nj, p0) with p0 the global pair index of the segment start.
SEGS = []
_p = 0
for _i in range(F - 1):
    _W = F - 1 - _i
    _j0 = 0
    while _j0 < _W:
        _room = 8 - (_p % 8)
        _nj = min(_room, _W - _j0)
        SEGS.append((_i, _j0, _nj, _p))
        _p += _nj
        _j0 += _nj
NWIN = (P + WPAIR - 1) // WPAIR
WINDOWS = [[] for _ in range(NWIN)]
for _s in SEGS:
    WINDOWS[_s[3] // WPAIR].append(_s)


def _wspan(w):
    """Written col extent of window w's dense tiles."""
    lo = w * WPAIR
    hi = min(P, lo + WPAIR)
    return (hi - lo) * D


STAGE = "full"  # bisect knob: gather | fact | inter | full


def build_nc(use_gpsimd_tt=False, stage=None):
    stage = STAGE if stage is None else stage
    nc = bacc.Bacc(None, target_bir_lowering=False)

    idx_d = nc.dram_tensor("idx", [BPC, F], mybir.dt.int32, kind="ExternalInput")
    tab_d = nc.dram_tensor("tab", [CARD * F, ROW], BF16, kind="ExternalInput")
    w1s_d = nc.dram_tensor("w1s", [128, D], BF16, kind="ExternalInput")
    w2sgn_d = nc.dram_tensor("w2sgn", [128, 32], BF16, kind="ExternalInput")
    b1c_d = nc.dram_tensor("b1c", [128, 1], FP32, kind="ExternalInput")
    linb_d = nc.dram_tensor("linb", [128, 1], FP32, kind="ExternalInput")
    y_d = nc.dram_tensor("y", [BPC, 1], FP32, kind="ExternalOutput")

    with TileContext(nc) as tc:
        with (
            tc.tile_pool(name="const", bufs=1) as cpool,
            tc.tile_pool(name="gath", bufs=2) as gpool,
            tc.tile_pool(name="fact", bufs=2) as fpool,
            tc.tile_pool(name="inter", bufs=3) as ipool,
            tc.tile_pool(name="hs", bufs=3) as hpool,
            tc.tile_pool(name="slsb", bufs=3) as slpool,
            tc.tile_pool(name="bm", bufs=2) as bmpool,
            tc.tile_pool(name="small", bufs=4) as smpool,
            tc.tile_pool(name="php", bufs=2, space="PSUM") as php,
            tc.tile_pool(name="psp", bufs=2, space="PSUM") as psp,
        ):
            # ---------------- constants / weights ----------------
            ident = cpool.tile([128, 128], BF16)
            make_identity(nc, ident[:])
            # dummy transpose: syncs PE with identity's producer once (real
            # transposes carry a single wait)
            warm = php.tile([64, 64], BF16, tag="hp")
            nc.tensor.transpose(warm[:], ident[0:64, 0:64], ident[0:64, 0:64])

            idx_sb = cpool.tile([128, G * F], mybir.dt.int32)
            nc.sync.dma_start(
                out=idx_sb[:].rearrange("p (g f) -> p g f", g=G),
                in_=idx_d[:].rearrange("(g p) f -> p g f", g=G),
            )

            w1s = cpool.tile([128, D], BF16)
            nc.sync.dma_start(out=w1s[:], in_=w1s_d[:])
            w2sgn = cpool.tile([128, 32], BF16)
            nc.sync.dma_start(out=w2sgn[:], in_=w2sgn_d[:])
            b1c = cpool.tile([128, 1], FP32)
            nc.sync.dma_start(out=b1c[:], in_=b1c_d[:])
            linb = cpool.tile([128, 1], FP32)
            nc.sync.dma_start(out=linb[:], in_=linb_d[:])
            ones_c = cpool.tile([128, 32], BF16)
            nc.vector.memset(ones_c[:], 0.0)
            nc.vector.memset(ones_c[:, 0:1], 1.0)

            # ---------------- main loop over 128-batch groups ----------------
            for g in range(G):
                gt = gpool.tile([128, F * ROW], BF16, tag="gt")
                for f in range(F):
                    nc.gpsimd.indirect_dma_start(
                        out=gt[:, f * ROW:(f + 1) * ROW],
                        out_offset=None,
                        in_=tab_d[:],
                        in_offset=bass.IndirectOffsetOnAxis(
                            ap=idx_sb[:, g * F + f:g * F + f + 1], axis=0
                        ),
                    )

                # linear term: fp32 view of the packed lin_w bits (col 32/33)
                gt_f32 = gt[:].bitcast(FP32).rearrange(
                    "p (f e) -> p f e", e=ROW // 2)
                lin_g = smpool.tile([128, 1], FP32, tag="lin")
                nc.vector.tensor_reduce(
                    out=lin_g[:],
                    in_=gt_f32[:, :, (ROW // 2) - 1:ROW // 2].rearrange(
                        "p f e -> p (f e)"),
                    axis=mybir.AxisListType.X, op=mybir.AluOpType.add,
                )
                lin_t = smpool.tile([128, 1], FP32, tag="lint")
                nc.vector.tensor_tensor(
                    out=lin_t[:], in0=lin_g[:], in1=linb[:],
                    op=mybir.AluOpType.add,
                )

                if stage == "gather":
                    nc.sync.dma_start(
                        out=y_d[g * 128:(g + 1) * 128, :], in_=lin_t[:])
                    continue

                # factors^T: [(d, half), (field, batch64)]
                fact = fpool.tile([128, F * D], BF16, tag="fact")
                tmpb = fpool.tile([64, F * D], BF16, tag="tmpb")
                for fb in range(0, F, 8):
                    nf = min(8, F - fb)
                    tpa = php.tile([64, 8, D], BF16, tag="hp")
                    tpb = php.tile([64, 8, D], BF16, tag="hp")
                    for j in range(nf):
                        f = fb + j
                        nc.tensor.transpose(
                            tpa[:, j, :],
                            gt[0:64, f * ROW:f * ROW + D],
                            ident[0:64, 0:64],
                        )
                        nc.tensor.transpose(
                            tpb[:, j, :],
                            gt[64:128, f * ROW:f * ROW + D],
                            ident[64:128, 64:128],
                        )
                    nc.scalar.activation(
                        out=fact[0:64, fb * D:(fb + nf) * D],
                        in_=tpa[:, 0:nf, :],
                        func=mybir.ActivationFunctionType.Copy,
                    )
                    nc.vector.tensor_copy(
                        tmpb[:, fb * D:(fb + nf) * D], tpb[:, 0:nf, :]
                    )
                nc.sync.dma_start(out=fact[64:128, :], in_=tmpb[:])

                if stage == "fact":
                    fsum = smpool.tile([128, 1], FP32, tag="fs")
                    nc.vector.tensor_reduce(
                        out=fsum[:], in_=fact[:],
                        axis=mybir.AxisListType.X, op=mybir.AluOpType.add)
                    nc.sync.dma_start(
                        out=y_d[g * 128:(g + 1) * 128, :], in_=fsum[:])
                    continue

                lgt_bm = bmpool.tile([128, P], FP32, tag="lgt")
                one_bm = bmpool.tile([128, P], FP32, tag="one")

                # pair-product tiles, one per block, (j, b) element order
                inters = {}

                def make_inter(i, fact=fact):
                    W = F - 1 - i
                    t = ipool.tile([128, W, D], BF16, tag="inter")
                    eng = nc.gpsimd if (use_gpsimd_tt and i % 3 == 2) \
                        else nc.vector
                    eng.tensor_tensor(
                        out=t[:],
                        in0=fact[:, i * D:(i + 1) * D]
                        .rearrange("p (o b) -> p o b", o=1)
                        .to_broadcast([128, W, D]),
                        in1=fact[:, (i + 1) * D:F * D]
                        .rearrange("p (j b) -> p j b", b=D),
                        op=mybir.AluOpType.mult,
                    )
                    return t

                if stage == "inter":
                    t0 = make_inter(0)
                    isum = smpool.tile([128, 1], FP32, tag="is")
                    nc.vector.tensor_reduce(
                        out=isum[:], in_=t0[:].rearrange("p j b -> p (j b)"),
                        axis=mybir.AxisListType.X, op=mybir.AluOpType.add)
                    nc.sync.dma_start(
                        out=y_d[g * 128:(g + 1) * 128, :], in_=isum[:])
                    continue

                prev = None  # (window index, hs tile)
                for w, wsegs in enumerate(WINDOWS):
                    span = _wspan(w)
                    hp = php.tile([128, 1024], FP32, tag="hp")
                    sl = psp.tile([128, 1024], FP32, tag="sl")
                    for (i, j0, nj, p0) in wsegs:
                        if i not in inters:
                            inters[i] = make_inter(i)
                            for dead in [d for d in inters if d < i - 1]:
                                del inters[dead]
                        it = inters[i]
                        N = nj * D
                        c0 = (p0 % WPAIR) * D
                        rhs = it[:, j0:j0 + nj, :]
                        rhs_bj = rhs.rearrange("p j b -> p b j")
                        # h' halves (col groups 0-1 / 2-3)
                        nc.tensor.matmul(
                            hp[0:64, c0:c0 + N], w1s[0:64, :], rhs[0:64],
                            start=True, stop=True,
                        )
                        nc.tensor.matmul(
                            hp[64:128, c0:c0 + N], w1s[64:128, :], rhs[64:128],
                            start=True, stop=True, tile_position=(64, 64),
                        )
                        if stage == "mm":
                            continue
                        # intersum strips, (b, j)-ordered cols
                        nc.tensor.matmul(
                            sl[64:96, c0:c0 + N], ones_c[0:64, :],
                            rhs_bj[0:64],
                            start=True, stop=True, tile_position=(0, 64),
                        )
                        nc.tensor.matmul(
                            sl[0:32, c0:c0 + N], ones_c[64:128, :],
                            rhs_bj[64:128],
                            start=True, stop=True, tile_position=(64, 0),
                        )
                    # logits for the previous window's hs (pipelined)
                    if prev is not None and stage in ("full", "lgt"):
                        pw, phs = prev
                        for (i, j0, nj, p0) in WINDOWS[pw]:
                            N = nj * D
                            c0 = (p0 % WPAIR) * D
                            hseg = phs[:, c0:c0 + N].rearrange(
                                "p (j b) -> p b j", b=D)
                            nc.tensor.matmul(
                                sl[96:128, c0:c0 + N], w2sgn[0:64, :],
                                hseg[0:64],
                                start=True, stop=True, tile_position=(0, 96),
                            )
                            nc.tensor.matmul(
                                sl[32:64, c0:c0 + N], w2sgn[64:128, :],
                                hseg[64:128],
                                start=True, stop=True, tile_position=(64, 32),
                            )
                    # relu drain -> hs (bf16); 2-of-3 windows on ACT
                    hs = hpool.tile([128, 1024], BF16, tag="hs")
                    if w % 3 != 2:
                        nc.scalar.activation(
                            out=hs[:, 0:span], in_=hp[:, 0:span],
                            func=mybir.ActivationFunctionType.Relu,
                            bias=b1c[:, 0:1],
                        )
                    else:
                        nc.vector.tensor_scalar(
                            out=hs[:, 0:span], in0=hp[:, 0:span],
                            scalar1=b1c[:, 0:1], scalar2=0.0,
                            op0=mybir.AluOpType.add, op1=mybir.AluOpType.max,
                        )
                    if stage == "mm":
                        prev = (w, hs)
                        continue
                    # slab drain (this window's sums + prev window's logits)
                    slsb = slpool.tile([128, 1024], FP32, tag="slsb")
                    pspan = (_wspan(prev[0])
                             if prev is not None and stage in ("full", "lgt")
                             else 0)
                    if span == pspan:
                        if w % 3 == 1:
                            nc.vector.tensor_copy(
                                slsb[:, 0:span], sl[:, 0:span])
                        else:
                            nc.scalar.activation(
                                out=slsb[:, 0:span], in_=sl[:, 0:span],
                                func=mybir.ActivationFunctionType.Copy,
                            )
                    else:
                        # first/last windows: sums and logits spans differ
                        for lo, hi, sp in ((0, 32, span), (32, 64, pspan),
                                           (64, 96, span), (96, 128, pspan)):
                            if sp:
                                nc.scalar.activation(
                                    out=slsb[lo:hi, 0:sp], in_=sl[lo:hi, 0:sp],
                                    func=mybir.ActivationFunctionType.Copy,
                                )
                    # compaction DMAs: sums of this window, logits of prev
                    for (i, j0, nj, p0) in wsegs:
                        c0 = (p0 % WPAIR) * D
                        s3 = slsb[:, c0:c0 + nj * D].rearrange(
                            "p (b j) -> p b j", j=nj)
                        nc.sync.dma_start(
                            out=one_bm[0:64, p0:p0 + nj], in_=s3[64:65])
                        nc.sync.dma_start(
                            out=one_bm[64:128, p0:p0 + nj], in_=s3[0:1])
                    if prev is not None and stage in ("full", "lgt"):
                        for (i, j0, nj, p0) in WINDOWS[prev[0]]:
                            c0 = (p0 % WPAIR) * D
                            s3 = slsb[:, c0:c0 + nj * D].rearrange(
                                "p (b j) -> p b j", j=nj)
                            nc.sync.dma_start(
                                out=lgt_bm[0:64, p0:p0 + nj], in_=s3[96:97])
                            nc.sync.dma_start(
                                out=lgt_bm[64:128, p0:p0 + nj], in_=s3[32:33])
                    prev = (w, hs)

                if stage in ("mm", "strips"):
                    pw, phs = prev
                    probe = smpool.tile([128, 1], FP32, tag="pr")
                    if stage == "mm":
                        nc.vector.tensor_reduce(
                            out=probe[:], in_=phs[:, 0:_wspan(pw)],
                            axis=mybir.AxisListType.X, op=mybir.AluOpType.add)
                    else:
                        nc.vector.tensor_reduce(
                            out=probe[:], in_=one_bm[:],
                            axis=mybir.AxisListType.X, op=mybir.AluOpType.add)
                    nc.sync.dma_start(
                        out=y_d[g * 128:(g + 1) * 128, :], in_=probe[:])
                    continue

                # tail: logits of the final window
                pw, phs = prev
                sl = psp.tile([128, 1024], FP32, tag="sl")
                for (i, j0, nj, p0) in WINDOWS[pw]:
                    N = nj * D
                    c0 = (p0 % WPAIR) * D
                    hseg = phs[:, c0:c0 + N].rearrange("p (j b) -> p b j", b=D)
                    nc.tensor.matmul(
                        sl[96:128, c0:c0 + N], w2sgn[0:64, :], hseg[0:64],
                        start=True, stop=True, tile_position=(0, 96),
                    )
                    nc.tensor.matmul(
                        sl[32:64, c0:c0 + N], w2sgn[64:128, :], hseg[64:128],
                        start=True, stop=True, tile_position=(64, 32),
                    )
                span = _wspan(pw)
                slsb = slpool.tile([128, 1024], FP32, tag="slsb")
                for lo, hi in ((32, 64), (96, 128)):
                    nc.scalar.activation(
                        out=slsb[lo:hi, 0:span], in_=sl[lo:hi, 0:span],
                        func=mybir.ActivationFunctionType.Copy,
                    )
                for (i, j0, nj, p0) in WINDOWS[pw]:
                    c0 = (p0 % WPAIR) * D
                    s3 = slsb[:, c0:c0 + nj * D].rearrange(
                        "p (b j) -> p b j", j=nj)
                    nc.sync.dma_start(
                        out=lgt_bm[0:64, p0:p0 + nj], in_=s3[96:97])
                    nc.sync.dma_start(
                        out=lgt_bm[64:128, p0:p0 + nj], in_=s3[32:33])

                if stage == "lgt":
                    probe2 = smpool.tile([128, 1], FP32, tag="pr2")
                    nc.vector.tensor_reduce(
                        out=probe2[:], in_=lgt_bm[:],
                        axis=mybir.AxisListType.X, op=mybir.AluOpType.add)
                    nc.sync.dma_start(
                        out=y_d[g * 128:(g + 1) * 128, :], in_=probe2[:])
                    continue

                # softmax + attended + linear
                ex = bmpool.tile([128, P], FP32, tag="ex")
                zsum = smpool.tile([128, 1], FP32, tag="z")
                nc.scalar.activation(
                    out=ex[:], in_=lgt_bm[:],
                    func=mybir.ActivationFunctionType.Exp,
                )
                nc.vector.tensor_reduce(
                    out=zsum[:], in_=ex[:],
                    axis=mybir.AxisListType.X, op=mybir.AluOpType.add,
                )
                wex = bmpool.tile([128, P], FP32, tag="wex")
                num = smpool.tile([128, 1], FP32, tag="num")
                nc.vector.tensor_tensor(
                    out=wex[:], in0=ex[:], in1=one_bm[:],
                    op=mybir.AluOpType.mult,
                )
                nc.vector.tensor_reduce(
                    out=num[:], in_=wex[:],
                    axis=mybir.AxisListType.X, op=mybir.AluOpType.add,
                )
                rz = smpool.tile([128, 1], FP32, tag="rz")
                nc.vector.reciprocal(rz[:], zsum[:])
                att = smpool.tile([128, 1], FP32, tag="att")
                nc.vector.tensor_tensor(
                    out=att[:], in0=num[:], in1=rz[:], op=mybir.AluOpType.mult
                )
                yg = smpool.tile([128, 1], FP32, tag="yg")
                nc.vector.tensor_tensor(
                    out=yg[:], in0=att[:], in1=lin_t[:], op=mybir.AluOpType.add
                )
                nc.sync.dma_start(out=y_d[g * 128:(g + 1) * 128, :], in_=yg[:])

    nc.compile()
    return nc


_CACHE = {}


def _pack_inputs(x, emb, W1, b1, w2, b2, lin_w, lin_b):
    idx = (x.astype(np.int64) + (np.arange(F, dtype=np.int64) * CARD)[None, :])
    idx = idx.astype(np.int32)

    emb_bf = emb.astype(ml_dtypes.bfloat16)
    lin_u16 = np.ascontiguousarray(
        lin_w.astype(np.float32).reshape(-1, 1)).view(np.uint16)
    tab = np.concatenate(
        [emb_bf.view(np.uint16), lin_u16], axis=1).view(ml_dtypes.bfloat16)
    tab = np.ascontiguousarray(tab)

    w2f = w2.reshape(D)
    w1p = (W1 * np.abs(w2f)[None, :]).astype(ml_dtypes.bfloat16)
    w1s = np.ascontiguousarray(np.vstack([w1p, w1p]))          # [128, 64]
    sgn = np.zeros((D, 32), np.float32)
    sgn[:, 0] = np.sign(w2f)
    sgn = sgn.astype(ml_dtypes.bfloat16)
    w2sgn = np.ascontiguousarray(np.vstack([sgn, sgn]))        # [128, 32]
    b1p = (b1.reshape(D) * np.abs(w2f)).astype(np.float32).reshape(D, 1)
    b1c = np.ascontiguousarray(np.vstack([b1p, b1p]))          # [128, 1]
    linb = np.broadcast_to(
        lin_b.reshape(1, 1), (128, 1)).astype(np.float32).copy()
    return idx, tab, w1s, w2sgn, b1c, linb


def kernel(x, emb, W1, b1, w2, b2, lin_w, lin_b):
    x = np.asarray(x)
    emb = np.asarray(emb, dtype=np.float32)
    W1 = np.asarray(W1, dtype=np.float32)
    b1 = np.asarray(b1, dtype=np.float32)
    w2 = np.asarray(w2, dtype=np.float32)
    lin_w = np.asarray(lin_w, dtype=np.float32)
    lin_b = np.asarray(lin_b, dtype=np.float32)

    idx, tab, w1s, w2sgn, b1c, linb = _pack_inputs(
        x, emb, W1, b1, w2, b2, lin_w, lin_b)

    if "nc" not in _CACHE:
        _CACHE["nc"] = build_nc()
    nc = _CACHE["nc"]

    in_maps = []
    for c in range(NCORES):
        in_maps.append({
            "idx": np.ascontiguousarray(idx[c * BPC:(c + 1) * BPC]),
            "tab": tab,
            "w1s": w1s,
            "w2sgn": w2sgn,
            "b1c": b1c,
            "linb": linb,
        })

    _CACHE["last_in_maps"] = in_maps
    res = run_bass_kernel_spmd(nc, in_maps, core_ids=list(range(NCORES)))
    outs = [res.results[c]["y"] for c in range(NCORES)]
    return np.concatenate(outs, axis=0).astype(np.float32)


if __name__ == "__main__":
    sys.path.insert(0, "/root/problem")
    import reference

    inputs = {k: np.asarray(v) for k, v in reference.setup_inputs().items()}
    y = kernel(**inputs)
    print(y.shape, y.dtype, y[:4, 0])
